# revision 13
# baseline (speedup 1.0000x reference)
"""Trainium2 Bass kernel for CapsDecorrelationNormalization (IterNorm).

Reference math (x: [B=128, CIN=32, COUT=128, ATOM=64] fp32):
  mean over (B, COUT, ATOM) per CIN; c = centered flattened [N, CIN];
  sigma = c^T c / (N-1);  W = newton_schulz_inv_sqrt(sigma, 5 iters);
  out = (c @ W) reshaped back * gamma + beta.

Strategy (8 NeuronCores, data-parallel over batch):
  - Each core owns 16 batches = [512, 8192] fp32 (16 MB), kept resident in SBUF.
  - Pass 1: PE-transpose 128-wide f-chunks of 4-batch-stacked [128, 8192]
    tiles, cast to bf16, and accumulate Gram matrix + per-cin sums via PE
    matmuls with an appended ones-column (block-diagonal in 4 batch groups).
  - AllReduce the packed [32, 33] (Gram | sums) across the 8 cores (~4 KB).
  - Replicated on-core: sigma from Gram/sums, Newton-Schulz (5 iters, fp32
    32x32 matmuls), fold gamma/beta/mean-correction into a per-partition
    scale/bias, replicate W across the 4 batch groups.
  - Pass 2: apply W with 4 concurrent tile_position matmuls (f32r), fused
    scale/bias on the PSUM->SBUF copy (in-place into the resident x tiles),
    DMA out.
"""

import numpy as np

B, CIN, COUT, ATOM = 128, 32, 128, 64
F = COUT * ATOM            # 8192
N_CORES = 8
BL = B // N_CORES          # 16 batches per core
BG = 4                     # batches stacked per 128-partition tile
NT = BL // BG              # 4 stacked tiles per core
ROWS = BG * CIN            # 128 partitions per stacked tile
NCHUNK = F // 128          # 64 transpose chunks per stacked tile
N_GLOBAL = float(B * F)    # 1048576 (norm_dim in the reference)
ITER_NUM = 5

_CACHE = {}


def _patch_tile_drain():
    """walrus rejects >1 sem wait on the kernel-tail Drain; spread the
    global-clock waits across preceding SP NOPs instead."""
    import concourse.tile as _tile
    from concourse.vector_clock import ScopedClock as _ScopedClock

    if getattr(_tile.TileContext, "_drain_patched", False):
        return

    def _patched(self, tick_clock, wait_clock):
        nops = [self.nc.sync.nop(nofuse=True) for _ in range(64)]
        drain_inst = self.nc.sync.drain()
        wait_clock.add_sem_waits(
            drain_inst.ins, _ScopedClock({None: tick_clock.global_clock})
        )
        si = drain_inst.ins.sync_info
        if si is not None and len(si.on_wait) > 1:
            assert self.sems is not None
            any_sem = next(iter(self.sems.allocated().values()))
            w = si.on_wait
            k = 0
            while len(w) > 1 and k < len(nops):
                tgt = nops[k]
                tgt._wait_ge(any_sem, 0)          # seed sync_info
                tgt.ins.sync_info.on_wait.pop()   # drop the seed
                tgt.ins.sync_info.on_wait.append(w.pop())
                k += 1
        self.nc.all_engine_barrier()
        assert self.sems is not None
        popped = self.nc._tile_sem_poison_stack.pop()
        assert popped is self._sem_poison
        self.nc.clear_and_free_semaphores(list(self.sems.allocated().values()))
        self.nc.all_engine_barrier()

    _tile.TileContext._drain_and_barrier = _patched
    _tile.TileContext._drain_patched = True


def _split_waits(nc, mybir, limit=1):
    """walrus allows very few sem waits per engine instruction on this
    build; hoist extras onto same-engine NOPs inserted just before."""
    import bass_rust
    for fn in nc.m.functions:
        for bb in fn.blocks:
            insts = bb.instructions
            k = 0
            while k < len(insts):
                inst = insts[k]
                si = inst.sync_info
                nw = len(si.on_wait) if si is not None else 0
                if nw > limit:
                    extras = [si.on_wait.pop() for _ in range(nw - limit)]
                    for w in extras:
                        nop = mybir.InstNoOp(
                            name=f"I-waitsplit-{nc.next_id()}", ins=[], outs=[]
                        )
                        nop.engine = inst.engine
                        nop.sync_info = bass_rust.SyncInfo(
                            on_wait=[w], on_update=[]
                        )
                        nc.register_instruction(nop)
                        insts.insert(k, nop)
                        k += 1
                k += 1


def _build_nc():
    import concourse.bass as bass
    import concourse.tile as tile
    from concourse import mybir
    from concourse.masks import make_identity

    _patch_tile_drain()

    f32 = mybir.dt.float32
    f32r = mybir.dt.float32r
    bf16 = mybir.dt.bfloat16

    nc = bass.Bass(num_devices=N_CORES)
    x_d = nc.declare_dram_parameter("x", [BL * CIN, F], f32r, isOutput=False)
    g_d = nc.declare_dram_parameter("gamma", [CIN, 1], f32, isOutput=False)
    b_d = nc.declare_dram_parameter("beta", [CIN, 1], f32, isOutput=False)
    o_d = nc.declare_dram_parameter("out", [BL * CIN, F], f32r, isOutput=True)

    FQ = F // 4  # DMA-in granularity (1 MB per [128, 2048] quarter)

    with tile.TileContext(nc) as tc:
        with tc.tile_pool(name="xs", bufs=1) as xs_pool, \
             tc.tile_pool(name="setup", bufs=1) as setup, \
             tc.tile_pool(name="stage", bufs=1) as stage_pool, \
             tc.tile_pool(name="gram", bufs=1, space="PSUM") as gram_pool, \
             tc.tile_pool(name="trp", bufs=3, space="PSUM") as trp_pool, \
             tc.tile_pool(name="newt", bufs=1) as newt, \
             tc.tile_pool(name="dram", bufs=1, space="DRAM") as dram:

            # ---------- setup ----------
            id128 = setup.tile([128, 128], f32)
            make_identity(nc, id128)
            id32 = id128[0:32, 0:32]
            id128r = setup.tile([128, 128], f32r)
            nc.vector.tensor_copy(out=id128r, in_=id128)

            ones32 = setup.tile([32, 32], f32)
            nc.vector.memset(ones32, 1.0)

            # irep[k, 32a+j] = I[k, j]  (identity replicated 4x along free)
            irep = setup.tile([32, 128], f32)
            for a in range(4):
                nc.vector.tensor_copy(out=irep[:, 32 * a:32 * a + 32], in_=id32)

            gb32 = setup.tile([32, 2], f32)
            nc.sync.dma_start(out=gb32[:, 0:1], in_=g_d[:, :])
            nc.sync.dma_start(out=gb32[:, 1:2], in_=b_d[:, :])

            # bf16 staging tiles for transposed chunks; col 128 = ones
            stages = []
            for i in range(4):
                st = stage_pool.tile([128, 132], bf16, tag=f"stage{i}", name=f"stage{i}")
                nc.vector.memset(st[:, 128:132], 1.0)
                stages.append(st)

            # resident input tiles (16 MB)
            xs = [xs_pool.tile([ROWS, F], f32r, tag=f"xs{t}", name=f"xs{t}") for t in range(NT)]

            # ---------- pass 1: load + Gram/sums ----------
            gram = gram_pool.tile([128, 132], f32)  # cols 0:128 gram, 128 sums

            for t in range(NT):
                for q in range(4):
                    nc.sync.dma_start(
                        out=xs[t][:, q * FQ:(q + 1) * FQ],
                        in_=x_d[t * ROWS:(t + 1) * ROWS, q * FQ:(q + 1) * FQ],
                    )

            mm_i = 0
            for t in range(NT):
                for c in range(NCHUNK):
                    src = xs[t][:, c * 128:(c + 1) * 128]
                    trp = trp_pool.tile([128, 128], f32r, tag="trp")
                    nc.tensor.transpose(trp, src, id128r)
                    st = stages[c % 4]
                    if c % 2 == 0:
                        nc.scalar.activation(
                            out=st[:, 0:128], in_=trp,
                            func=mybir.ActivationFunctionType.Copy,
                        )
                    else:
                        nc.vector.tensor_copy(out=st[:, 0:128], in_=trp)
                    nc.tensor.matmul(
                        gram[:, 0:129], lhsT=st[:, 0:128], rhs=st[:, 0:129],
                        start=(mm_i == 0), stop=(mm_i == NT * NCHUNK - 1),
                    )
                    mm_i += 1

            # ---------- fold 4 batch-group blocks + pack ----------
            # evacuate gram PSUM -> SBUF, then fold blocks a=1..3 onto block 0
            # with accumulate-DMAs (engines cannot cross partition bases)
            pack = newt.tile([32, 33], f32)
            gsb = newt.tile([128, 132], f32)
            nc.scalar.activation(out=gsb, in_=gram[:, :],
                                 func=mybir.ActivationFunctionType.Copy)
            for a in range(1, 4):
                pr = slice(32 * a, 32 * a + 32)
                nc.gpsimd.dma_start(
                    out=gsb[0:32, 0:32], in_=gsb[pr, 32 * a:32 * a + 32],
                    accum_op=mybir.AluOpType.add,
                )
                nc.gpsimd.dma_start(
                    out=gsb[0:32, 128:129], in_=gsb[pr, 128:129],
                    accum_op=mybir.AluOpType.add,
                )
            nc.vector.tensor_copy(out=pack[:, 0:32], in_=gsb[0:32, 0:32])
            nc.vector.tensor_copy(out=pack[:, 32:33], in_=gsb[0:32, 128:129])

            cc_in = dram.tile([32, 33], f32)
            cc_out = dram.tile([32, 33], f32)
            nc.sync.dma_start(out=cc_in[:], in_=pack[:, :])
            nc.gpsimd.collective_compute(
                "AllReduce", mybir.AluOpType.add,
                replica_groups=[list(range(N_CORES))],
                ins=[cc_in.opt()], outs=[cc_out.opt()],
            )
            stats = newt.tile([32, 33], f32)
            nc.sync.dma_start(out=stats[:, :], in_=cc_out[:])

            # ---------- sigma, trace, Newton-Schulz (fp32, replicated) ----------
            with tc.tile_pool(name="nps", bufs=2, space="PSUM") as ps:
                # mean = s / N
                m32 = newt.tile([32, 1], f32)
                nc.scalar.mul(out=m32, in_=stats[:, 32:33], mul=1.0 / N_GLOBAL)

                # s s^T outer product (via s^T row vector)
                stp = ps.tile([32, 32], f32, tag="nps")
                nc.tensor.transpose(stp[0:1, 0:32], stats[:, 32:33], id32)
                st_sb = newt.tile([1, 32], f32)
                nc.vector.tensor_copy(out=st_sb, in_=stp[0:1, 0:32])
                outer = ps.tile([32, 32], f32, tag="nps")
                nc.tensor.matmul(outer[:, :], lhsT=st_sb, rhs=st_sb,
                                 start=True, stop=True)

                # sigma = G/(N-1) - ssT/(N(N-1))
                c1 = 1.0 / (N_GLOBAL - 1.0)
                c2 = -1.0 / (N_GLOBAL * (N_GLOBAL - 1.0))
                sig = newt.tile([32, 32], f32)
                otmp = newt.tile([32, 32], f32)
                nc.vector.tensor_scalar(out=sig, in0=stats[:, 0:32], scalar1=c1,
                                        scalar2=None, op0=mybir.AluOpType.mult)
                nc.vector.tensor_scalar(out=otmp, in0=outer[:, :], scalar1=c2,
                                        scalar2=None, op0=mybir.AluOpType.mult)
                nc.vector.tensor_add(sig, sig, otmp)

                # trace -> all partitions
                dtmp = newt.tile([32, 32], f32)
                nc.vector.tensor_mul(dtmp, sig, id32)
                dcol = newt.tile([32, 1], f32)
                nc.vector.reduce_sum(out=dcol, in_=dtmp, axis=mybir.AxisListType.X)
                trp2 = ps.tile([32, 32], f32, tag="nps")
                nc.tensor.matmul(trp2[:, 0:1], lhsT=ones32, rhs=dcol,
                                 start=True, stop=True)
                itr = newt.tile([32, 1], f32)
                nc.vector.reciprocal(out=itr, in_=trp2[:, 0:1])
                rst = newt.tile([32, 1], f32)
                nc.scalar.activation(out=rst, in_=itr,
                                     func=mybir.ActivationFunctionType.Sqrt)

                sigN = newt.tile([32, 32], f32)
                nc.vector.tensor_scalar_mul(out=sigN, in0=sig, scalar1=itr)

                i15 = newt.tile([32, 32], f32)
                nc.vector.tensor_scalar_mul(out=i15, in0=id32, scalar1=1.5)

                # p_{k+1} = p (1.5 I - 0.5 p^2 sigN)
                p_cur = newt.tile([32, 32], f32)
                nc.vector.tensor_copy(out=p_cur, in_=id32)
                for k in range(ITER_NUM):
                    a_ps = ps.tile([32, 32], f32, tag="nps")
                    nc.tensor.matmul(a_ps[:, :], lhsT=p_cur, rhs=p_cur,
                                     start=True, stop=True)
                    a_sb = newt.tile([32, 32], f32, tag="a_sb")
                    nc.vector.tensor_copy(out=a_sb, in_=a_ps[:, :])
                    d_ps = ps.tile([32, 32], f32, tag="nps")
                    nc.tensor.matmul(d_ps[:, :], lhsT=a_sb, rhs=sigN,
                                     start=True, stop=True)
                    e_sb = newt.tile([32, 32], f32, tag="e_sb")
                    nc.vector.tensor_scalar(out=e_sb, in0=d_ps[:, :], scalar1=-0.5,
                                            scalar2=None, op0=mybir.AluOpType.mult)
                    nc.vector.tensor_add(e_sb, e_sb, i15)
                    pn_ps = ps.tile([32, 32], f32, tag="nps")
                    nc.tensor.matmul(pn_ps[:, :], lhsT=p_cur, rhs=e_sb,
                                     start=True, stop=True)
                    p_nxt = newt.tile([32, 32], f32, tag=f"p{(k + 1) % 2}")
                    nc.vector.tensor_copy(out=p_nxt, in_=pn_ps[:, :])
                    p_cur = p_nxt

                # W = p * rsqrt(trace)
                w_sb = newt.tile([32, 32], f32)
                nc.vector.tensor_scalar_mul(out=w_sb, in0=p_cur, scalar1=rst)

                # wrep[k, 32a+j] = W[k, j]
                wrep = newt.tile([32, 128], f32)
                for a in range(4):
                    nc.vector.tensor_copy(out=wrep[:, 32 * a:32 * a + 32], in_=w_sb)

                # w4[32a+j, n] = W[j, n] (W symmetric)
                w4_ps = ps.tile([128, 32], f32, tag="nps128")
                nc.tensor.matmul(w4_ps[:, :], lhsT=wrep, rhs=id32,
                                 start=True, stop=True)
                w4 = newt.tile([128, 32], f32)
                nc.vector.tensor_copy(out=w4, in_=w4_ps[:, :])
                # block-diagonal W4f[32a+i, 32a+j] = W[i, j], zeros elsewhere
                w4z = newt.tile([128, 128], f32)
                nc.vector.memset(w4z, 0.0)
                w4f = newt.tile([128, 128], f32r)
                nc.vector.tensor_copy(out=w4f, in_=w4z)
                for a in range(4):
                    pr4 = slice(32 * a, 32 * a + 32)
                    nc.vector.tensor_copy(out=w4f[pr4, 32 * a:32 * a + 32],
                                          in_=w4[pr4, :])

                # gamma/beta replicated to 128 partitions
                gb_ps = ps.tile([128, 2], f32, tag="nps128")
                nc.tensor.matmul(gb_ps[:, :], lhsT=irep, rhs=gb32,
                                 start=True, stop=True)
                gb128 = newt.tile([128, 2], f32)
                nc.vector.tensor_copy(out=gb128, in_=gb_ps[:, :])

                # mW replicated: mw128[32a+j] = sum_i W[i, j] m[i]
                mw_ps = ps.tile([128, 1], f32, tag="nps128")
                nc.tensor.matmul(mw_ps[:, :], lhsT=wrep, rhs=m32,
                                 start=True, stop=True)
                bias128 = newt.tile([128, 1], f32)
                nc.vector.tensor_mul(bias128, mw_ps[:, :], gb128[:, 0:1])
                nc.vector.tensor_tensor(out=bias128, in0=gb128[:, 1:2],
                                        in1=bias128, op=mybir.AluOpType.subtract)

            # ---------- pass 2: apply + store ----------
            with tc.tile_pool(name="apply", bufs=2, space="PSUM") as ap_pool:
                NSL = F // 512  # 16 slices per tile
                for t in range(NT):
                    for fs in range(NSL):
                        sl = slice(fs * 512, (fs + 1) * 512)
                        ap_ps = ap_pool.tile([128, 512], f32, tag="ap")
                        nc.tensor.matmul(
                            ap_ps[:, :], lhsT=w4f, rhs=xs[t][:, sl],
                            start=True, stop=True,
                        )
                        if fs % 2 == 0:
                            nc.scalar.activation(
                                out=xs[t][:, sl], in_=ap_ps[:, :],
                                func=mybir.ActivationFunctionType.Identity,
                                scale=gb128[:, 0:1], bias=bias128,
                            )
                        else:
                            nc.vector.tensor_scalar(
                                out=xs[t][:, sl], in0=ap_ps[:, :],
                                scalar1=gb128[:, 0:1], scalar2=bias128,
                                op0=mybir.AluOpType.mult,
                                op1=mybir.AluOpType.add,
                            )
                    for h in range(2):
                        hs = slice(h * (F // 2), (h + 1) * (F // 2))
                        nc.sync.dma_start(
                            out=o_d[t * ROWS:(t + 1) * ROWS, hs],
                            in_=xs[t][:, hs],
                        )
    _split_waits(nc, mybir)
    return nc


def _get_nc():
    if "nc" not in _CACHE:
        _CACHE["nc"] = _build_nc()
    return _CACHE["nc"]


def kernel(x, gamma, beta):
    from concourse.bass_utils import run_bass_kernel_spmd

    nc = _get_nc()
    x = np.ascontiguousarray(np.asarray(x, dtype=np.float32))
    g = np.asarray(gamma, dtype=np.float32).reshape(CIN, 1)
    b = np.asarray(beta, dtype=np.float32).reshape(CIN, 1)
    in_maps = []
    for i in range(N_CORES):
        shard = x[i * BL:(i + 1) * BL].reshape(BL * CIN, F)
        in_maps.append({"x": shard, "gamma": g, "beta": b})
    res = run_bass_kernel_spmd(nc, in_maps, list(range(N_CORES)))
    out = np.concatenate(
        [res.results[i]["out"].reshape(BL, CIN, COUT, ATOM) for i in range(N_CORES)],
        axis=0,
    )
    return out
